# revision 13
# baseline (speedup 1.0000x reference)
"""CORAL loss kernel for Trainium2 (8 NeuronCores, Bass/Tile).

Strategy (data-parallel over bz, per sharding hint):
  - Shard features [32, 4096, 256] along bz: 4 batch elements per core.
  - Host appends a ones column to each row (d -> d+1) so the device gets
    xaug[b] = [X | 1] and a single DMA produces both matmul operands.
  - Per batch element b on device: stream xaug[b] ([n=4096, 257]) through
    SBUF in [128, 8, 257] chunks. Accumulate S = X^T X on the PE into PSUM
    (two 128-row chunks); the ones column makes column d of each PSUM chunk
    the colsum of X for free. The per-batch mean row m is extracted by
    PE-transposing the two colsum columns, and a rank-1 accumulating matmul
    adds -n * m ⊗ m into PSUM, yielding (n-1)*cov in place. A scaled copy
    writes cov = (S - n m m^T)/(n-1) to SBUF, DMA'd out.
  - Host gathers the small per-batch means [32, 256] and covs [32, 256, 256]
    and does the tiny masked pairwise reduction in float64 (exact mirror of
    the reference math).

Hardware note: a PE Matmult/Ldweights can carry at most ONE semaphore wait,
so the structure keeps every instruction at <=1 wait: the ones column comes
in with the same DMA as the data (no second producer), each batch's PSUM
slot is claimed by a tiny const-only matmul (absorbing the slot-release
wait), and small SBUF pools have one slot per batch element (no reuse).
"""

import sys

import numpy as np

if "/opt/trn_rl_repo" not in sys.path:
    sys.path.insert(0, "/opt/trn_rl_repo")

import concourse.bass as bass
import concourse.mybir as mybir
import concourse.tile as tile
from concourse.masks import make_identity

BZ, N, D = 32, 4096, 256
NCORES = 8
BPC = BZ // NCORES  # batch elements per core
P = 128  # partitions


def build_nc(bpc=BPC, n=N, d=D, kc=8, xp_bufs=None):
    """Build the per-core Bass module: covs/means for `bpc` batch elements.

    Input "x" is the host-augmented [bpc, n, d+1] tensor ([X | ones]).
    """
    assert n % P == 0 and d == 2 * P
    kt = n // P  # k-tiles of 128 rows
    assert kt % kc == 0
    nchunk = kt // kc  # DMA chunks per batch element
    if xp_bufs is None:
        # One slot per chunk-load: x-tile slots are never reused, so x DMAs
        # never need a slot-release wait (DMAs also carry at most one wait).
        xp_bufs = bpc * nchunk

    nc = bass.Bass(trn_type="TRN2")
    f32 = mybir.dt.float32
    x = nc.dram_tensor("x", [bpc, n, d + 1], f32, kind="ExternalInput")
    covs = nc.dram_tensor("covs", [bpc, d, d], f32, kind="ExternalOutput")
    means = nc.dram_tensor("means", [bpc, d], f32, kind="ExternalOutput")

    copy_fn = mybir.ActivationFunctionType.Copy

    with tile.TileContext(nc) as tc:
        with (
            tc.tile_pool(name="xp", bufs=xp_bufs) as xp,
            tc.tile_pool(name="covp", bufs=bpc) as covp,
            tc.tile_pool(name="rowp", bufs=bpc) as rowp,
            tc.tile_pool(name="smallp", bufs=bpc) as smallp,
            tc.tile_pool(name="constp", bufs=1) as constp,
            tc.tile_pool(name="psp", bufs=2, space="PSUM") as psp,
            tc.tile_pool(name="pstp", bufs=2, space="PSUM") as pstp,
        ):
            ident = constp.tile([P, P], f32)
            make_identity(nc, ident)

            def emit_kloop(b):
                # PSUM accumulators for the two 128-row chunks of S = X^T X.
                # Column d holds colsum(X) via the ones column of xaug.
                ps0 = psp.tile([P, d + 1], f32, tag="ps0", name=f"ps0_{b}")
                ps1 = psp.tile([P, d + 1], f32, tag="ps1", name=f"ps1_{b}")
                # Claim the PSUM slots with a tiny const-only matmul so the
                # slot-release wait lands here, not on the first real matmul
                # (PE instructions support only one wait). The garbage value
                # is cleared by start=True on the first real matmul.
                nc.tensor.matmul(
                    ps0[0:1, 0:1], ident[:, 0:1], ident[:, 0:1],
                    start=True, stop=True, skip_group_check=True,
                )
                nc.tensor.matmul(
                    ps1[0:1, 0:1], ident[:, 0:1], ident[:, 0:1],
                    start=True, stop=True, skip_group_check=True,
                )
                for c in range(nchunk):
                    xt = xp.tile([P, kc, d + 1], f32, tag="xt", name=f"xt_{b}_{c}")
                    src = x[b].rearrange("(c k p) e -> c p k e", p=P, k=kc)[c]
                    nc.sync.dma_start(out=xt[:, :, :], in_=src)
                    for k in range(kc):
                        kk = c * kc + k
                        nc.tensor.matmul(
                            ps0[:, :], xt[:, k, 0:P], xt[:, k, :],
                            start=(kk == 0), stop=(kk == kt - 1),
                        )
                        nc.tensor.matmul(
                            ps1[:, :], xt[:, k, P:d], xt[:, k, :],
                            start=(kk == 0), stop=(kk == kt - 1),
                        )
                return ps0, ps1

            def emit_epilogue(b, ps0, ps1):
                mcol = smallp.tile([P, 2], f32, tag="mcol", name=f"mcol_{b}")
                nc.scalar.activation(mcol[:, 0:1], ps0[:, d : d + 1], copy_fn, scale=1.0 / n)
                nc.scalar.activation(mcol[:, 1:2], ps1[:, d : d + 1], copy_fn, scale=1.0 / n)
                # Transpose each [128,1] mean column to a [1,128] row
                # separately (partition accesses must start at 0).
                psr0 = pstp.tile([1, P], f32, tag="psr0", name=f"psr0_{b}")
                psr1 = pstp.tile([1, P], f32, tag="psr1", name=f"psr1_{b}")
                # Claim the transpose PSUM banks (see emit_kloop).
                nc.tensor.matmul(
                    psr0[0:1, 0:1], ident[:, 0:1], ident[:, 0:1],
                    start=True, stop=True, skip_group_check=True,
                )
                nc.tensor.matmul(
                    psr1[0:1, 0:1], ident[:, 0:1], ident[:, 0:1],
                    start=True, stop=True, skip_group_check=True,
                )
                nc.tensor.transpose(psr0[0:1, :], mcol[:, 0:1], ident[:, :])
                nc.tensor.transpose(psr1[0:1, :], mcol[:, 1:2], ident[:, :])
                mrow = rowp.tile([1, d], f32, tag="mrow", name=f"mrow_{b}")
                nc.scalar.copy(mrow[0:1, 0:P], psr0[0:1, :])
                nc.scalar.copy(mrow[0:1, P:d], psr1[0:1, :])
                nrow = rowp.tile([1, d], f32, tag="nrow", name=f"nrow_{b}")
                nc.scalar.mul(nrow[0:1, :], mrow[0:1, :], -float(n))
                # S - n m m^T via rank-1 accumulate into PSUM (cols 0:d only).
                # start=False: keeps has_written set, so this adds onto S.
                nc.tensor.matmul(
                    ps0[:, 0:d], nrow[0:1, 0:P], mrow[0:1, :], start=False, stop=True,
                    skip_group_check=True,
                )
                nc.tensor.matmul(
                    ps1[:, 0:d], nrow[0:1, P:d], mrow[0:1, :], start=False, stop=True,
                    skip_group_check=True,
                )
                cov = covp.tile([P, 2, d], f32, tag="cov", name=f"cov_{b}")
                inv = 1.0 / (n - 1)
                nc.scalar.activation(cov[:, 0, :], ps0[:, 0:d], copy_fn, scale=inv)
                nc.scalar.activation(cov[:, 1, :], ps1[:, 0:d], copy_fn, scale=inv)
                # SWDGE (gpsimd) for outputs: keeps them off the 8 HWDGE
                # semaphore lanes used by the x loads, so no lane-FIFO wait
                # stacks on top of the data wait (one-wait limit per DMA).
                nc.gpsimd.dma_start(
                    out=covs[b].rearrange("(c p) e -> p c e", c=2), in_=cov[:, :, :]
                )
                nc.gpsimd.dma_start(out=means[b : b + 1, :], in_=mrow[0:1, :])

            # Sequential emission (kloop b, epilogue b, kloop b+1, ...).
            # This keeps every PSUM slot-claim's ACT-release wait transitively
            # implied by an earlier PE wait (one-wait-per-instruction limit).
            # The PE stalls briefly in each epilogue, but DMA is the critical
            # path and the x loads never wait on the PE (dedicated slots).
            for b in range(bpc):
                ps0, ps1 = emit_kloop(b)
                emit_epilogue(b, ps0, ps1)

    _install_drain_split(nc)
    return nc


def _split_drain_waits(bir, max_waits=1):
    """Split any Drain carrying more than `max_waits` sem waits into a chain
    of single-wait Drains (the HW sync-wait table is tiny; Tile's kernel-tail
    drain waits on every active sem lane at once)."""
    for fn in bir["functions"]:
        for blk in fn["blocks"]:
            out = []
            changed = False
            for inst in blk["instructions"]:
                waits = (inst.get("sync_info") or {}).get("on_wait") or []
                if inst.get("opcode") == "Drain" and len(waits) > max_waits:
                    changed = True
                    for wi in range(0, len(waits) - max_waits):
                        clone = {
                            **inst,
                            "name": f"{inst['name']}_w{wi}",
                            "sync_info": {
                                "on_wait": [waits[wi]],
                                "on_update": [],
                            },
                        }
                        out.append(clone)
                    inst = {
                        **inst,
                        "sync_info": {
                            **inst["sync_info"],
                            "on_wait": waits[len(waits) - max_waits :],
                        },
                    }
                out.append(inst)
            if changed:
                blk["instructions"] = out
    return bir


def _install_drain_split(nc):
    import orjson

    raw = nc.to_json_bytes

    def patched():
        return orjson.dumps(_split_drain_waits(orjson.loads(raw())))

    nc.to_json_bytes = patched


_NC_CACHE = {}


def _get_nc():
    key = (BPC, N, D)
    if key not in _NC_CACHE:
        _NC_CACHE[key] = build_nc()
    return _NC_CACHE[key]


def augment_ones(feats, bpc, n, d):
    """[cores, bpc, n, d] view -> per-core [bpc, n, d+1] with ones column."""
    out = np.empty((feats.shape[0], bpc, n, d + 1), dtype=np.float32)
    out[..., :d] = feats
    out[..., d] = 1.0
    return out


def coral_from_stats(means, covs, domains, d=D):
    """Masked pairwise CORAL reduction from per-batch stats (float64)."""
    bz = means.shape[0]
    m = means.astype(np.float64)
    ms = (m * m).sum(1)
    md = (ms[:, None] + ms[None, :] - 2.0 * (m @ m.T)) / d
    v = covs.astype(np.float64).reshape(bz, -1)
    cs = (v * v).sum(1)
    g = v @ v.T
    cd = (cs[:, None] + cs[None, :] - 2.0 * g) / (d * d)
    upper = np.triu(np.ones((bz, bz), dtype=bool), k=1)
    mask = upper & (np.asarray(domains)[:, None] != np.asarray(domains)[None, :])
    loss = np.where(mask, md + cd, 0.0).sum()
    num = int(mask.sum())
    if num > 1:
        loss = loss / num
    return np.float32(loss)


def kernel(features, domains, _trace=False):
    from concourse import bass_utils

    feats = np.asarray(features)
    assert feats.shape == (BZ, N, D)
    xaug = augment_ones(
        np.asarray(feats, dtype=np.float32).reshape(NCORES, BPC, N, D), BPC, N, D
    )
    nc = _get_nc()
    in_maps = [{"x": xaug[c]} for c in range(NCORES)]
    res = bass_utils.run_bass_kernel_spmd(
        nc, in_maps, core_ids=list(range(NCORES)), trace=_trace
    )
    covs = np.concatenate([r["covs"] for r in res.results], axis=0)
    means = np.concatenate([r["means"] for r in res.results], axis=0)
    out = coral_from_stats(means, covs, domains)
    if _trace:
        return out, res
    return out


# revision 23
# speedup vs baseline: 2.3228x; 2.3228x over previous
"""CORAL loss kernel for Trainium2 (8 NeuronCores, Bass/Tile).

Strategy (data-parallel over bz, per sharding hint):
  - Shard features [32, 4096, 256] along bz: 4 batch elements per core.
  - Host casts features to fp16 and appends a ones column (d -> d+1), so the
    device reads half the bytes and the PE runs single-pass matmuls (fp32
    matmuls lower to two LO/HI passes on TRN2 and are ~4x slower). PSUM
    accumulation stays fp32; the resulting loss error is ~1e-5 relative
    (the CORAL loss is a large average, so per-element fp16 noise washes
    out; measured 1.8e-6 for the fp32 variant, same order for fp16).
  - Per batch element b on device: stream xaug[b] ([n=4096, 257]) through
    SBUF in [128, 16, 257] chunks. Accumulate S = X^T X on the PE into PSUM
    (two 128-row chunks); the ones column makes column d of each PSUM chunk
    the colsum of X for free. The per-batch mean row m is extracted by
    PE-transposing the two colsum columns, and a rank-1 accumulating matmul
    adds -n * m ⊗ m into PSUM, yielding (n-1)*cov in place. A scaled copy
    writes cov = (S - n m m^T)/(n-1) to SBUF, DMA'd out.
  - Host gathers the small per-batch means [32, 256] and covs [32, 256, 256]
    and does the tiny masked pairwise reduction in float64 (exact mirror of
    the reference math).

Hardware note: most instructions here can carry at most ONE semaphore wait
(PE Matmult/Ldweights, DMA descriptors), so the structure keeps every
instruction at <=1 wait: the ones column comes in with the same DMA as the
data (no second producer), each PSUM bank is claimed by a tiny const-only
matmul (absorbing the slot-release wait), a PE "fence" matmul at the end of
each epilogue makes the cov-copy completion transitively observed by the PE
(so later claims need no ACT wait), small SBUF pools have one slot per batch
element, and x loads get dedicated slots + one DMA per HWDGE lane.
"""

import sys

import numpy as np

if "/opt/trn_rl_repo" not in sys.path:
    sys.path.insert(0, "/opt/trn_rl_repo")

import concourse.bass as bass
import concourse.mybir as mybir
import concourse.tile as tile
from concourse.masks import make_identity
from concourse.tile_rust import add_dep_helper

BZ, N, D = 32, 4096, 256
NCORES = 8
BPC = BZ // NCORES  # batch elements per core
P = 128  # partitions


def build_nc(bpc=BPC, n=N, d=D, kc=16, xp_bufs=None, pipelined=True):
    """Build the per-core Bass module: covs/means for `bpc` batch elements.

    Input "x" is the host-prepared fp16 [bpc, n, d+1] tensor ([X | ones]).
    """
    assert n % P == 0 and d == 2 * P
    kt = n // P  # k-tiles of 128 rows
    assert kt % kc == 0
    nchunk = kt // kc  # DMA chunks per batch element
    if xp_bufs is None:
        # One slot per chunk-load: x-tile slots are never reused, so x DMAs
        # never need a slot-release wait (DMAs also carry at most one wait).
        xp_bufs = bpc * nchunk

    nc = bass.Bass(trn_type="TRN2")
    f32 = mybir.dt.float32
    f16 = mybir.dt.float16
    x = nc.dram_tensor("x", [bpc, n, d + 1], f16, kind="ExternalInput")
    covs = nc.dram_tensor("covs", [bpc, d, d], f32, kind="ExternalOutput")
    means = nc.dram_tensor("means", [bpc, d], f32, kind="ExternalOutput")

    copy_fn = mybir.ActivationFunctionType.Copy

    with tile.TileContext(nc) as tc:
        with (
            tc.tile_pool(name="xp", bufs=xp_bufs) as xp,
            tc.tile_pool(name="covp", bufs=bpc) as covp,
            tc.tile_pool(name="rowp", bufs=bpc) as rowp,
            tc.tile_pool(name="smallp", bufs=bpc) as smallp,
            tc.tile_pool(name="constp", bufs=1) as constp,
            tc.tile_pool(name="psp", bufs=2, space="PSUM") as psp,
            tc.tile_pool(name="pstp", bufs=bpc, space="PSUM") as pstp,
        ):
            ident = constp.tile([P, P], f32)
            make_identity(nc, ident)

            def claim(pstile, after=None):
                # Tiny const-only matmul whose only job is to carry the PSUM
                # bank slot-release wait (one-wait-per-PE-instruction limit).
                # Garbage value; cleared by start=True of the first real use.
                inst = nc.tensor.matmul(
                    pstile[0:1, 0:1], ident[:, 0:1], ident[:, 0:1],
                    start=True, stop=True, skip_group_check=True,
                )
                if after is not None:
                    # Pin the claim after the fence of the bank's previous
                    # user (same engine, order-only): its ACT-release wait is
                    # then implied by the fence's wait and elided, leaving
                    # only the PE bank-drain wait.
                    add_dep_helper(inst.ins, after.ins, sync=False,
                                   reason="psum claim after fence")
                return inst

            def emit_kloop(b, fence=None):
                # PSUM accumulators for the two 128-row chunks of S = X^T X.
                # Column d holds colsum(X) via the ones column of xaug.
                ps0 = psp.tile([P, d + 1], f32, tag="ps0", name=f"ps0_{b}")
                ps1 = psp.tile([P, d + 1], f32, tag="ps1", name=f"ps1_{b}")
                claim(ps0, after=fence)
                claim(ps1, after=fence)
                for c in range(nchunk):
                    xt = xp.tile([P, kc, d + 1], f16, tag="xt", name=f"xt_{b}_{c}")
                    src = x[b].rearrange("(c k p) e -> c p k e", p=P, k=kc)[c]
                    nc.sync.dma_start(out=xt[:, :, :], in_=src)
                    for k in range(kc):
                        kk = c * kc + k
                        nc.tensor.matmul(
                            ps0[:, :], xt[:, k, 0:P], xt[:, k, :],
                            start=(kk == 0), stop=(kk == kt - 1),
                        )
                        nc.tensor.matmul(
                            ps1[:, :], xt[:, k, P:d], xt[:, k, :],
                            start=(kk == 0), stop=(kk == kt - 1),
                        )
                return ps0, ps1

            def emit_epilogue(b, ps0, ps1):
                mcol = smallp.tile([P, 2], f32, tag="mcol", name=f"mcol_{b}")
                nc.scalar.activation(mcol[:, 0:1], ps0[:, d : d + 1], copy_fn, scale=1.0 / n)
                nc.scalar.activation(mcol[:, 1:2], ps1[:, d : d + 1], copy_fn, scale=1.0 / n)
                # Transpose each [128,1] mean column to a [1,128] row
                # separately (partition accesses must start at 0). Both land
                # in one [1,256] PSUM row; one slot per batch element, so the
                # bank is never reused and no claim/release waits are needed.
                psr = pstp.tile([1, d], f32, tag="psr", name=f"psr_{b}")
                nc.tensor.transpose(psr[0:1, 0:P], mcol[:, 0:1], ident[:, :])
                nc.tensor.transpose(psr[0:1, P:d], mcol[:, 1:2], ident[:, :])
                mrow = rowp.tile([1, d], f32, tag="mrow", name=f"mrow_{b}")
                nc.scalar.copy(mrow[0:1, 0:P], psr[0:1, 0:P])
                nc.scalar.copy(mrow[0:1, P:d], psr[0:1, P:d])
                # fp16 copies for the rank-1 correction matmul (keeps the PE
                # single-pass; the correction is ~1e-5 of the cov scale).
                mrow16 = rowp.tile([1, d], f16, tag="mrow16", name=f"mrow16_{b}")
                nrow16 = rowp.tile([1, d], f16, tag="nrow16", name=f"nrow16_{b}")
                nc.scalar.copy(mrow16[0:1, :], mrow[0:1, :])
                nc.scalar.activation(nrow16[0:1, :], mrow[0:1, :], copy_fn, scale=-float(n))
                # S - n m m^T via rank-1 accumulate into PSUM (cols 0:d only).
                # start=False: keeps has_written set, so this adds onto S.
                nc.tensor.matmul(
                    ps0[:, 0:d], nrow16[0:1, 0:P], mrow16[0:1, :], start=False,
                    stop=True, skip_group_check=True,
                )
                nc.tensor.matmul(
                    ps1[:, 0:d], nrow16[0:1, P:d], mrow16[0:1, :], start=False,
                    stop=True, skip_group_check=True,
                )
                cov = covp.tile([P, 2, d], f32, tag="cov", name=f"cov_{b}")
                inv = 1.0 / (n - 1)
                nc.scalar.activation(cov[:, 0, :], ps0[:, 0:d], copy_fn, scale=inv)
                nc.scalar.activation(cov[:, 1, :], ps1[:, 0:d], copy_fn, scale=inv)
                # SWDGE (gpsimd) for outputs: keeps them off the 8 HWDGE
                # semaphore lanes used by the x loads, so no lane-FIFO wait
                # stacks on top of the data wait (one-wait limit per DMA).
                nc.gpsimd.dma_start(
                    out=covs[b].rearrange("(c p) e -> p c e", c=2), in_=cov[:, :, :]
                )
                nc.gpsimd.dma_start(out=means[b : b + 1, :], in_=mrow[0:1, :])
                # PE fence: a tiny matmul reading the cov tile, so the PE's
                # observed ACT clock passes the cov copies. Later PSUM bank
                # claims then need no explicit ACT wait (it is implied),
                # keeping them at one wait each. Reuses psr's bank.
                # Reads chunk 1 (the LAST cov copy in ACT order) so the
                # implied ACT clock covers both copies.
                return nc.tensor.matmul(
                    psr[0:1, 0:1], cov[:, 1, 0:1], cov[:, 1, 0:1],
                    start=True, stop=True, skip_group_check=True,
                )

            if pipelined:
                # One-batch software pipeline: epilogue(b) is emitted after
                # kloop(b+1), so the PE stream never stalls on the epilogue's
                # serial ACT chain. The fence op keeps later claims at one
                # wait despite the reordering.
                prev = None
                fences = {}
                for b in range(bpc):
                    cur = emit_kloop(b, fence=fences.get(b - 2))
                    if prev is not None:
                        fences[b - 1] = emit_epilogue(b - 1, *prev)
                    prev = cur
                emit_epilogue(bpc - 1, *prev)
            else:
                fence = None
                for b in range(bpc):
                    ps0, ps1 = emit_kloop(b, fence=fence)
                    fence = emit_epilogue(b, ps0, ps1)

    _install_drain_split(nc)
    return nc


def _split_drain_waits(bir, max_waits=1):
    """Split any Drain carrying more than `max_waits` sem waits into a chain
    of single-wait Drains (the HW sync-wait table is tiny; Tile's kernel-tail
    drain waits on every active sem lane at once)."""
    for fn in bir["functions"]:
        for blk in fn["blocks"]:
            out = []
            changed = False
            for inst in blk["instructions"]:
                waits = (inst.get("sync_info") or {}).get("on_wait") or []
                if inst.get("opcode") == "Drain" and len(waits) > max_waits:
                    changed = True
                    for wi in range(0, len(waits) - max_waits):
                        clone = {
                            **inst,
                            "name": f"{inst['name']}_w{wi}",
                            "sync_info": {
                                "on_wait": [waits[wi]],
                                "on_update": [],
                            },
                        }
                        out.append(clone)
                    inst = {
                        **inst,
                        "sync_info": {
                            **inst["sync_info"],
                            "on_wait": waits[len(waits) - max_waits :],
                        },
                    }
                out.append(inst)
            if changed:
                blk["instructions"] = out
    return bir


def _install_drain_split(nc):
    import orjson

    raw = nc.to_json_bytes

    def patched():
        return orjson.dumps(_split_drain_waits(orjson.loads(raw())))

    nc.to_json_bytes = patched


_NC_CACHE = {}


def _get_nc():
    key = (BPC, N, D)
    if key not in _NC_CACHE:
        _NC_CACHE[key] = build_nc()
    return _NC_CACHE[key]


def augment_ones_f16(feats, bpc, n, d):
    """[cores, bpc, n, d] fp32 -> per-core fp16 [bpc, n, d+1] with ones."""
    out = np.empty((feats.shape[0], bpc, n, d + 1), dtype=np.float16)
    out[..., :d] = feats
    out[..., d] = 1.0
    return out


def coral_from_stats(means, covs, domains, d=D):
    """Masked pairwise CORAL reduction from per-batch stats (float64)."""
    bz = means.shape[0]
    m = means.astype(np.float64)
    ms = (m * m).sum(1)
    md = (ms[:, None] + ms[None, :] - 2.0 * (m @ m.T)) / d
    v = covs.astype(np.float64).reshape(bz, -1)
    cs = (v * v).sum(1)
    g = v @ v.T
    cd = (cs[:, None] + cs[None, :] - 2.0 * g) / (d * d)
    upper = np.triu(np.ones((bz, bz), dtype=bool), k=1)
    mask = upper & (np.asarray(domains)[:, None] != np.asarray(domains)[None, :])
    loss = np.where(mask, md + cd, 0.0).sum()
    num = int(mask.sum())
    if num > 1:
        loss = loss / num
    return np.float32(loss)


def kernel(features, domains, _trace=False):
    from concourse import bass_utils

    feats = np.asarray(features)
    assert feats.shape == (BZ, N, D)
    xaug = augment_ones_f16(
        np.asarray(feats, dtype=np.float32).reshape(NCORES, BPC, N, D), BPC, N, D
    )
    nc = _get_nc()
    in_maps = [{"x": xaug[c]} for c in range(NCORES)]
    res = bass_utils.run_bass_kernel_spmd(
        nc, in_maps, core_ids=list(range(NCORES)), trace=_trace
    )
    covs = np.concatenate([r["covs"] for r in res.results], axis=0)
    means = np.concatenate([r["means"] for r in res.results], axis=0)
    out = coral_from_stats(means, covs, domains)
    if _trace:
        return out, res
    return out


# revision 25
# speedup vs baseline: 2.7236x; 1.1726x over previous
"""CORAL loss kernel for Trainium2 (8 NeuronCores, Bass/Tile).

Strategy (data-parallel over bz, per sharding hint):
  - Shard features [32, 4096, 256] along bz: 4 batch elements per core.
  - Host casts features to fp16 and appends a ones column (d -> d+1), so the
    device reads half the bytes and the PE runs single-pass matmuls (fp32
    matmuls lower to two LO/HI passes on TRN2 and are ~4x slower). PSUM
    accumulation stays fp32; the loss error from fp16 inputs is ~1e-6
    relative (the CORAL loss is a large average, so per-element quantization
    noise washes out; measured 2.7e-6 end to end).
  - Per batch element b on device: stream xaug[b] ([n=4096, 257]) through
    SBUF in [128, 16, 257] chunks. Accumulate G = Xaug^T Xaug on the PE into
    PSUM as two 128-row chunks [128, 257]; thanks to the ones column, column
    d of each chunk is colsum(X) for free. DVE copies PSUM to SBUF, one DMA
    writes the raw [128, 2, 257] block out. No other device math.
  - Host (float64): cov_b = (S_b - colsum_b ⊗ m_b)/(n-1), m_b = colsum_b/n,
    then the tiny masked pairwise CORAL reduction (exact mirror of the
    reference math). This is ~10 MFLOP on 8.4 MB of stats - gather-scale
    work, like the all-gather + replicated reduction in the sharding hint.

Hardware note: most instructions can carry at most ONE semaphore wait (PE
Matmult/Ldweights, DMA descriptors), so the structure keeps every
instruction at <=1 wait: the ones column arrives with the data DMA (single
producer per x tile), x tiles get dedicated SBUF slots (no reuse -> x DMAs
never wait), PSUM banks are claimed by a tiny const-only matmul pinned
(order-only dep) after the previous user's PE "fence", and the fence reads
the staged output tile so the DVE-release of the PSUM bank is transitively
implied. Tile's kernel-tail Drain (one wait per active sem lane) is split
into single-wait drains by a JSON post-pass.
"""

import sys

import numpy as np

if "/opt/trn_rl_repo" not in sys.path:
    sys.path.insert(0, "/opt/trn_rl_repo")

import concourse.bass as bass
import concourse.mybir as mybir
import concourse.tile as tile
from concourse.masks import make_identity
from concourse.tile_rust import add_dep_helper

BZ, N, D = 32, 4096, 256
NCORES = 8
BPC = BZ // NCORES  # batch elements per core
P = 128  # partitions


def build_nc(bpc=BPC, n=N, d=D, kc=16, xp_bufs=None):
    """Per-core Bass module: raw S blocks for `bpc` batch elements.

    Input "x": host-prepared fp16 [bpc, n, d+1] ([X | ones]).
    Output "outs": fp32 [bpc, 2, 128, d+1]; outs[b, c] = (Xaug^T Xaug) rows
    c*128..c*128+127, i.e. S chunks with the colsum in column d.
    """
    assert n % P == 0 and d == 2 * P
    kt = n // P  # k-tiles of 128 rows
    assert kt % kc == 0
    nchunk = kt // kc  # DMA chunks per batch element
    if xp_bufs is None:
        # One slot per chunk-load: x-tile slots are never reused, so x DMAs
        # never need a slot-release wait (DMAs also carry at most one wait).
        xp_bufs = bpc * nchunk

    nc = bass.Bass(trn_type="TRN2")
    f32 = mybir.dt.float32
    f16 = mybir.dt.float16
    x = nc.dram_tensor("x", [bpc, n, d + 1], f16, kind="ExternalInput")
    outs = nc.dram_tensor("outs", [bpc, 2, P, d + 1], f32, kind="ExternalOutput")

    with tile.TileContext(nc) as tc:
        with (
            tc.tile_pool(name="xp", bufs=xp_bufs) as xp,
            tc.tile_pool(name="op", bufs=bpc) as op,
            tc.tile_pool(name="constp", bufs=1) as constp,
            tc.tile_pool(name="psp", bufs=2, space="PSUM") as psp,
        ):
            ident = constp.tile([P, P], f16)
            make_identity(nc, ident)

            def claim(pstile, after=None):
                # Tiny const-only matmul whose only job is to carry the PSUM
                # bank slot-release wait (one-wait-per-PE-instruction limit).
                # Garbage value; cleared by start=True of the first real use.
                inst = nc.tensor.matmul(
                    pstile[0:1, 0:1], ident[:, 0:1], ident[:, 0:1],
                    start=True, stop=True, skip_group_check=True,
                )
                if after is not None:
                    # Pin the claim after the fence of the bank's previous
                    # user (same engine, order-only): the DVE-release wait is
                    # then implied by the fence's wait and elided, leaving
                    # only the PE bank-drain wait.
                    add_dep_helper(inst.ins, after.ins, sync=False,
                                   reason="psum claim after fence")
                return inst

            def emit_kloop(b, fence=None):
                # PSUM accumulators for the two 128-row chunks of Xaug^T Xaug.
                ps0 = psp.tile([P, d + 1], f32, tag="ps0", name=f"ps0_{b}")
                ps1 = psp.tile([P, d + 1], f32, tag="ps1", name=f"ps1_{b}")
                claim(ps0, after=fence)
                claim(ps1, after=fence)
                for c in range(nchunk):
                    xt = xp.tile([P, kc, d + 1], f16, tag="xt", name=f"xt_{b}_{c}")
                    src = x[b].rearrange("(c k p) e -> c p k e", p=P, k=kc)[c]
                    nc.sync.dma_start(out=xt[:, :, :], in_=src)
                    for k in range(kc):
                        kk = c * kc + k
                        nc.tensor.matmul(
                            ps0[:, :], xt[:, k, 0:P], xt[:, k, :],
                            start=(kk == 0), stop=(kk == kt - 1),
                        )
                        nc.tensor.matmul(
                            ps1[:, :], xt[:, k, P:d], xt[:, k, :],
                            start=(kk == 0), stop=(kk == kt - 1),
                        )
                return ps0, ps1

            def emit_epilogue(b, ps0, ps1):
                ot = op.tile([P, 2, d + 1], f32, tag="ot", name=f"ot_{b}")
                nc.vector.tensor_copy(ot[:, 0, :], ps0[:, :])
                nc.vector.tensor_copy(ot[:, 1, :], ps1[:, :])
                # SWDGE (gpsimd) for outputs: keeps them off the 8 HWDGE
                # semaphore lanes used by the x loads, so no lane-FIFO wait
                # stacks on top of the data wait (one-wait limit per DMA).
                nc.gpsimd.dma_start(
                    out=outs[b].rearrange("c p e -> p c e"), in_=ot[:, :, :]
                )
                # PE fence: reads the part of the staged tile written by the
                # LAST DVE copy, so the PE's observed DVE clock passes both
                # PSUM reads; the next claim of these banks then needs no
                # explicit DVE wait. Writes garbage into ps0 after its data
                # was staged - harmless.
                return nc.tensor.matmul(
                    ps0[0:1, 0:1], ot[:, 1, 0:1], ot[:, 1, 0:1],
                    start=True, stop=True, skip_group_check=True,
                )

            # One-batch software pipeline: epilogue(b) is emitted after
            # kloop(b+1) so the PE stream never stalls on the epilogue.
            prev = None
            fences = {}
            for b in range(bpc):
                cur = emit_kloop(b, fence=fences.get(b - 2))
                if prev is not None:
                    fences[b - 1] = emit_epilogue(b - 1, *prev)
                prev = cur
            emit_epilogue(bpc - 1, *prev)

    _install_drain_split(nc)
    return nc


def _split_drain_waits(bir, max_waits=1):
    """Split any Drain carrying more than `max_waits` sem waits into a chain
    of single-wait Drains (the HW sync-wait table is tiny; Tile's kernel-tail
    drain waits on every active sem lane at once)."""
    for fn in bir["functions"]:
        for blk in fn["blocks"]:
            out = []
            changed = False
            for inst in blk["instructions"]:
                waits = (inst.get("sync_info") or {}).get("on_wait") or []
                if inst.get("opcode") == "Drain" and len(waits) > max_waits:
                    changed = True
                    for wi in range(0, len(waits) - max_waits):
                        clone = {
                            **inst,
                            "name": f"{inst['name']}_w{wi}",
                            "sync_info": {
                                "on_wait": [waits[wi]],
                                "on_update": [],
                            },
                        }
                        out.append(clone)
                    inst = {
                        **inst,
                        "sync_info": {
                            **inst["sync_info"],
                            "on_wait": waits[len(waits) - max_waits :],
                        },
                    }
                out.append(inst)
            if changed:
                blk["instructions"] = out
    return bir


def _install_drain_split(nc):
    import orjson

    raw = nc.to_json_bytes

    def patched():
        return orjson.dumps(_split_drain_waits(orjson.loads(raw())))

    nc.to_json_bytes = patched


_NC_CACHE = {}


def _get_nc():
    key = (BPC, N, D)
    if key not in _NC_CACHE:
        _NC_CACHE[key] = build_nc()
    return _NC_CACHE[key]


def augment_ones_f16(feats, bpc, n, d):
    """[cores, bpc, n, d] fp32 -> per-core fp16 [bpc, n, d+1] with ones."""
    out = np.empty((feats.shape[0], bpc, n, d + 1), dtype=np.float16)
    out[..., :d] = feats
    out[..., d] = 1.0
    return out


def stats_from_raw(outs_blocks, n=N, d=D):
    """Device outs [bz, 2, 128, d+1] -> (means [bz,d], covs [bz,d,d]) f64."""
    bz = outs_blocks.shape[0]
    s_aug = outs_blocks.astype(np.float64).reshape(bz, d, d + 1)
    s = s_aug[:, :, :d]
    colsum = s_aug[:, :, d]
    m = colsum / n
    covs = (s - colsum[:, :, None] * m[:, None, :]) / (n - 1)
    return m, covs


def coral_from_stats(means, covs, domains, d=D):
    """Masked pairwise CORAL reduction from per-batch stats (float64)."""
    bz = means.shape[0]
    m = means.astype(np.float64)
    ms = (m * m).sum(1)
    md = (ms[:, None] + ms[None, :] - 2.0 * (m @ m.T)) / d
    v = covs.astype(np.float64).reshape(bz, -1)
    cs = (v * v).sum(1)
    g = v @ v.T
    cd = (cs[:, None] + cs[None, :] - 2.0 * g) / (d * d)
    upper = np.triu(np.ones((bz, bz), dtype=bool), k=1)
    mask = upper & (np.asarray(domains)[:, None] != np.asarray(domains)[None, :])
    loss = np.where(mask, md + cd, 0.0).sum()
    num = int(mask.sum())
    if num > 1:
        loss = loss / num
    return np.float32(loss)


def kernel(features, domains, _trace=False):
    from concourse import bass_utils

    feats = np.asarray(features)
    assert feats.shape == (BZ, N, D)
    xaug = augment_ones_f16(
        np.asarray(feats, dtype=np.float32).reshape(NCORES, BPC, N, D), BPC, N, D
    )
    nc = _get_nc()
    in_maps = [{"x": xaug[c]} for c in range(NCORES)]
    res = bass_utils.run_bass_kernel_spmd(
        nc, in_maps, core_ids=list(range(NCORES)), trace=_trace
    )
    blocks = np.concatenate([r["outs"] for r in res.results], axis=0)
    means, covs = stats_from_raw(blocks)
    out = coral_from_stats(means, covs, domains)
    if _trace:
        return out, res
    return out


# revision 28
# speedup vs baseline: 3.0876x; 1.1336x over previous
"""CORAL loss kernel for Trainium2 (8 NeuronCores, Bass/Tile).

Strategy (data-parallel over bz, per sharding hint):
  - Shard features [32, 4096, 256] along bz: 4 batch elements per core.
  - Host casts features to fp16 and appends a ones column (d -> d+1), so the
    device reads half the bytes and the PE runs single-pass matmuls (fp32
    matmuls lower to two LO/HI passes on TRN2 and are ~4x slower). PSUM
    accumulation stays fp32; the loss error from fp16 inputs is ~1e-6
    relative (the CORAL loss is a large average, so per-element quantization
    noise washes out; measured 2.7e-6 end to end).
  - Per batch element b on device: stream xaug[b] ([n=4096, 257]) through
    SBUF in [128, 16, 257] chunks. Accumulate G = Xaug^T Xaug on the PE into
    PSUM as two 128-row chunks [128, 257]; thanks to the ones column, column
    d of each chunk is colsum(X) for free. DVE copies PSUM to SBUF, one DMA
    writes the raw [128, 2, 257] block out. No other device math.
  - Host (float64): cov_b = (S_b - colsum_b ⊗ m_b)/(n-1), m_b = colsum_b/n,
    then the tiny masked pairwise CORAL reduction (exact mirror of the
    reference math). This is ~10 MFLOP on 8.4 MB of stats - gather-scale
    work, like the all-gather + replicated reduction in the sharding hint.

Hardware note: most instructions can carry at most ONE semaphore wait (PE
Matmult/Ldweights, DMA descriptors), so the structure keeps every
instruction at <=1 wait: the ones column arrives with the data DMA (single
producer per x tile), x tiles get dedicated SBUF slots (no reuse -> x DMAs
never wait), PSUM banks are claimed by a tiny const-only matmul pinned
(order-only dep) after the previous user's PE "fence", and the fence reads
the staged output tile so the DVE-release of the PSUM bank is transitively
implied. Tile's kernel-tail Drain (one wait per active sem lane) is split
into single-wait drains by a JSON post-pass.
"""

import sys

import numpy as np

if "/opt/trn_rl_repo" not in sys.path:
    sys.path.insert(0, "/opt/trn_rl_repo")

import concourse.bass as bass
import concourse.mybir as mybir
import concourse.tile as tile
from concourse.masks import make_identity
from concourse.tile_rust import add_dep_helper

BZ, N, D = 32, 4096, 256
NCORES = 8
BPC = BZ // NCORES  # batch elements per core
P = 128  # partitions


def build_nc(bpc=BPC, n=N, d=D, kc=16, xp_bufs=None):
    """Per-core Bass module: raw S blocks for `bpc` batch elements.

    Input "x": host-prepared fp16 [bpc, n, d+1] ([X | ones]).
    Output "outs": fp32 [bpc, 2, 128, d+1]; outs[b, c] = (Xaug^T Xaug) rows
    c*128..c*128+127, i.e. S chunks with the colsum in column d.
    """
    assert n % P == 0 and d == 2 * P
    kt = n // P  # k-tiles of 128 rows
    assert kt % kc == 0
    nchunk = kt // kc  # DMA chunks per batch element
    if xp_bufs is None:
        # One slot per chunk-load: x-tile slots are never reused, so x DMAs
        # never need a slot-release wait (DMAs also carry at most one wait).
        xp_bufs = bpc * nchunk

    nc = bass.Bass(trn_type="TRN2")
    f32 = mybir.dt.float32
    f16 = mybir.dt.float16
    x = nc.dram_tensor("x", [bpc, n, d + 1], f16, kind="ExternalInput")
    # Packed per-batch output: [S[0:128, 0:256] | colsum[0:128]] (257 cols)
    # ++ [S[128:256, 128:256] | colsum[128:256]] (129 cols). The lower-left
    # block of S is mirrored on the host (S is symmetric).
    w0, w1 = d + 1, d // 2 + 1
    outs = nc.dram_tensor("outs", [bpc, P, w0 + w1], f32, kind="ExternalOutput")

    with tile.TileContext(nc) as tc:
        with (
            tc.tile_pool(name="xp", bufs=xp_bufs) as xp,
            tc.tile_pool(name="op", bufs=bpc) as op,
            tc.tile_pool(name="constp", bufs=1) as constp,
            tc.tile_pool(name="psp", bufs=2, space="PSUM") as psp,
        ):
            ident = constp.tile([P, P], f16)
            make_identity(nc, ident)

            def claim(pstile, after=None):
                # Tiny const-only matmul whose only job is to carry the PSUM
                # bank slot-release wait (one-wait-per-PE-instruction limit).
                # Garbage value; cleared by start=True of the first real use.
                inst = nc.tensor.matmul(
                    pstile[0:1, 0:1], ident[:, 0:1], ident[:, 0:1],
                    start=True, stop=True, skip_group_check=True,
                )
                if after is not None:
                    # Pin the claim after the fence of the bank's previous
                    # user (same engine, order-only): the DVE-release wait is
                    # then implied by the fence's wait and elided, leaving
                    # only the PE bank-drain wait.
                    add_dep_helper(inst.ins, after.ins, sync=False,
                                   reason="psum claim after fence")
                return inst

            def emit_kloop(b, fence=None):
                # PSUM accumulators: ps0 = S rows 0:128 (all cols) + colsum,
                # ps1 = S rows 128:256, cols 128:256 only (symmetry) + colsum.
                ps0 = psp.tile([P, w0], f32, tag="ps0", name=f"ps0_{b}")
                ps1 = psp.tile([P, w1], f32, tag="ps1", name=f"ps1_{b}")
                claim(ps0, after=fence)
                claim(ps1, after=fence)
                for c in range(nchunk):
                    xt = xp.tile([P, kc, d + 1], f16, tag="xt", name=f"xt_{b}_{c}")
                    # Partition p holds kt consecutive rows of x[b]; any
                    # partition of the n rows into 128-row k-tiles is valid
                    # for sum_n x x^T, and consecutive rows give long
                    # contiguous DMA runs (full HBM bandwidth).
                    src = x[b].rearrange("(p k) e -> p k e", p=P)[
                        :, c * kc : (c + 1) * kc, :
                    ]
                    nc.sync.dma_start(out=xt[:, :, :], in_=src)
                    for k in range(kc):
                        kk = c * kc + k
                        nc.tensor.matmul(
                            ps0[:, :], xt[:, k, 0:P], xt[:, k, :],
                            start=(kk == 0), stop=(kk == kt - 1),
                        )
                        nc.tensor.matmul(
                            ps1[:, :], xt[:, k, P:d], xt[:, k, P : d + 1],
                            start=(kk == 0), stop=(kk == kt - 1),
                        )
                return ps0, ps1

            def emit_epilogue(b, ps0, ps1):
                ot = op.tile([P, w0 + w1], f32, tag="ot", name=f"ot_{b}")
                nc.vector.tensor_copy(ot[:, 0:w0], ps0[:, :])
                nc.vector.tensor_copy(ot[:, w0 : w0 + w1], ps1[:, :])
                # SWDGE (gpsimd) for outputs: keeps them off the 8 HWDGE
                # semaphore lanes used by the x loads, so no lane-FIFO wait
                # stacks on top of the data wait (one-wait limit per DMA).
                nc.gpsimd.dma_start(out=outs[b], in_=ot[:, :])
                # PE fence: reads the part of the staged tile written by the
                # LAST DVE copy, so the PE's observed DVE clock passes both
                # PSUM reads; the next claim of these banks then needs no
                # explicit DVE wait. Writes garbage into ps0 after its data
                # was staged - harmless.
                return nc.tensor.matmul(
                    ps0[0:1, 0:1], ot[:, w0 + w1 - 1 : w0 + w1], ot[:, w0 + w1 - 1 : w0 + w1],
                    start=True, stop=True, skip_group_check=True,
                )

            # One-batch software pipeline: epilogue(b) is emitted after
            # kloop(b+1) so the PE stream never stalls on the epilogue.
            prev = None
            fences = {}
            for b in range(bpc):
                cur = emit_kloop(b, fence=fences.get(b - 2))
                if prev is not None:
                    fences[b - 1] = emit_epilogue(b - 1, *prev)
                prev = cur
            emit_epilogue(bpc - 1, *prev)

    _install_drain_split(nc)
    return nc


def _split_drain_waits(bir, max_waits=1):
    """Split any Drain carrying more than `max_waits` sem waits into a chain
    of single-wait Drains (the HW sync-wait table is tiny; Tile's kernel-tail
    drain waits on every active sem lane at once)."""
    for fn in bir["functions"]:
        for blk in fn["blocks"]:
            out = []
            changed = False
            for inst in blk["instructions"]:
                waits = (inst.get("sync_info") or {}).get("on_wait") or []
                if inst.get("opcode") == "Drain" and len(waits) > max_waits:
                    changed = True
                    for wi in range(0, len(waits) - max_waits):
                        clone = {
                            **inst,
                            "name": f"{inst['name']}_w{wi}",
                            "sync_info": {
                                "on_wait": [waits[wi]],
                                "on_update": [],
                            },
                        }
                        out.append(clone)
                    inst = {
                        **inst,
                        "sync_info": {
                            **inst["sync_info"],
                            "on_wait": waits[len(waits) - max_waits :],
                        },
                    }
                out.append(inst)
            if changed:
                blk["instructions"] = out
    return bir


def _install_drain_split(nc):
    import orjson

    raw = nc.to_json_bytes

    def patched():
        return orjson.dumps(_split_drain_waits(orjson.loads(raw())))

    nc.to_json_bytes = patched


_NC_CACHE = {}


def _get_nc():
    key = (BPC, N, D)
    if key not in _NC_CACHE:
        _NC_CACHE[key] = build_nc()
    return _NC_CACHE[key]


def augment_ones_f16(feats, bpc, n, d):
    """[cores, bpc, n, d] fp32 -> per-core fp16 [bpc, n, d+1] with ones."""
    out = np.empty((feats.shape[0], bpc, n, d + 1), dtype=np.float16)
    out[..., :d] = feats
    out[..., d] = 1.0
    return out


def stats_from_raw(outs_blocks, n=N, d=D):
    """Device outs [bz, 128, 386] (packed, see build_nc) -> f64 stats."""
    bz = outs_blocks.shape[0]
    h = d // 2
    o = outs_blocks.astype(np.float64)
    s = np.empty((bz, d, d))
    s[:, :h, :] = o[:, :, 0:d]
    s[:, h:, h:] = o[:, :, d + 1 : d + 1 + h]
    s[:, h:, :h] = np.swapaxes(o[:, :, h:d], 1, 2)  # symmetry mirror
    colsum = np.concatenate([o[:, :, d], o[:, :, d + 1 + h]], axis=1)
    m = colsum / n
    covs = (s - colsum[:, :, None] * m[:, None, :]) / (n - 1)
    return m, covs


def coral_from_stats(means, covs, domains, d=D):
    """Masked pairwise CORAL reduction from per-batch stats (float64)."""
    bz = means.shape[0]
    m = means.astype(np.float64)
    ms = (m * m).sum(1)
    md = (ms[:, None] + ms[None, :] - 2.0 * (m @ m.T)) / d
    v = covs.astype(np.float64).reshape(bz, -1)
    cs = (v * v).sum(1)
    g = v @ v.T
    cd = (cs[:, None] + cs[None, :] - 2.0 * g) / (d * d)
    upper = np.triu(np.ones((bz, bz), dtype=bool), k=1)
    mask = upper & (np.asarray(domains)[:, None] != np.asarray(domains)[None, :])
    loss = np.where(mask, md + cd, 0.0).sum()
    num = int(mask.sum())
    if num > 1:
        loss = loss / num
    return np.float32(loss)


def kernel(features, domains, _trace=False):
    from concourse import bass_utils

    feats = np.asarray(features)
    assert feats.shape == (BZ, N, D)
    xaug = augment_ones_f16(
        np.asarray(feats, dtype=np.float32).reshape(NCORES, BPC, N, D), BPC, N, D
    )
    nc = _get_nc()
    in_maps = [{"x": xaug[c]} for c in range(NCORES)]
    res = bass_utils.run_bass_kernel_spmd(
        nc, in_maps, core_ids=list(range(NCORES)), trace=_trace
    )
    blocks = np.concatenate([r["outs"] for r in res.results], axis=0)
    means, covs = stats_from_raw(blocks)
    out = coral_from_stats(means, covs, domains)
    if _trace:
        return out, res
    return out
